# revision 19
# baseline (speedup 1.0000x reference)
"""Multi-head cross attention on 8 Trainium2 NeuronCores.

Sharding: core c = b*4 + g handles batch b (of 2) and head-group g (4 heads
of the 16).  Each core projects Q/K/V for its 4 heads, runs attention, and
computes partial output projections with its 256 rows of Wo split into two
128-row head-pair chunks; the host sums the 8 bf16 partials per batch (plus
bo and the bv@Wo term, exact because softmax rows sum to 1).

v3 schedule: one software-pipelined stream.  All input DMAs are issued up
front (weights, then xkv, then xq); KT matmuls chase the xkv tiles as they
land; V' runs for kv tiles 0-8 and the first two Q chunks, then attention
starts (~45us in).  Everything else - the remaining V' tiles, the remaining
QT chunks, and all output-projection tiles - runs as PE filler inside the
ACT-bound attention loop (exp gates it at ~1.2us per kv tile).  AV matmuls
lag the score matmuls by two kv tiles so the PSUM-accumulator handoff at
block boundaries never stalls the PE queue ahead of the exp stream.  The
output projection is split by head pair (one K=128 matmul per tile) so each
half runs as soon as its pair is normalized; only the last quarter's second
half remains as tail.  Row-sum reciprocals use the fast custom-DVE op.
"""

import sys

sys.path.insert(0, "/opt/trn_rl_repo")

import ml_dtypes
import numpy as np

BF16NP = ml_dtypes.bfloat16

B, SQ, SKV, D, H = 2, 2048, 2048, 1024, 16
DH = D // H          # 64
N_CORES = 8
G = 4                # head groups
HPG = H // G         # heads per group = 4
GC = HPG * DH        # group width = 256

_nc_cache = None


def _build_nc():
    import concourse.mybir as mybir
    import concourse.tile as tile
    from concourse import bacc

    F32 = mybir.dt.float32
    BF16 = mybir.dt.bfloat16
    AF = mybir.ActivationFunctionType
    MUL = mybir.AluOpType.mult

    nc = bacc.Bacc("TRN2", target_bir_lowering=False, debug=False,
                   num_devices=N_CORES)

    xqT_d = nc.dram_tensor("xqT", [D, SQ], BF16, kind="ExternalInput").ap()
    xkvT_d = nc.dram_tensor("xkvT", [D, SKV], BF16, kind="ExternalInput").ap()
    wq_d = nc.dram_tensor("wq", [D, GC], BF16, kind="ExternalInput").ap()
    wk_d = nc.dram_tensor("wk", [D, GC], BF16, kind="ExternalInput").ap()
    # Wv' with a zero column after each head's 64 (slots for the ones column)
    wvp_d = nc.dram_tensor("wvp", [D, HPG * 65], BF16, kind="ExternalInput").ap()
    wo_d = nc.dram_tensor("wo", [GC, D], BF16, kind="ExternalInput").ap()
    bq_d = nc.dram_tensor("bq2", [128, 2], F32, kind="ExternalInput").ap()
    bk_d = nc.dram_tensor("bk2", [128, 2], F32, kind="ExternalInput").ap()
    out0_d = nc.dram_tensor("out_p0", [SQ, D], BF16, kind="ExternalOutput").ap()
    out1_d = nc.dram_tensor("out_p1", [SQ, D], BF16, kind="ExternalOutput").ap()
    out_ds = [out0_d, out1_d]

    ND = D // 128        # 8 d-tiles (contraction over D)
    NJ = SKV // 128      # 16 kv tiles
    VW = HPG * 65        # 260, V' row width
    NJ_PRE = 10          # V' tiles computed before attention starts
    scale = 1.0 / float(np.sqrt(DH))

    with tile.TileContext(nc) as tc:
        with (
            tc.tile_pool(name="persist", bufs=1) as pp,
            tc.tile_pool(name="work", bufs=1) as wk_pool,
        ):
            # ---- persistent tiles -------------------------------------
            qt_sb = pp.tile([128, 2 * SQ], BF16, tag="qt_sb")
            kt_sb = pp.tile([128, 2 * SKV], BF16, tag="kt_sb")
            vp_sb = pp.tile([128, NJ * VW + 63], BF16, tag="vp_sb")
            o_sbA = pp.tile([128, 2 * 1024], BF16, tag="o_sbA")
            o_sbB = pp.tile([128, 2 * 1024], BF16, tag="o_sbB")
            bq_sb = pp.tile([128, 2], F32, tag="bq_sb")
            bk_sb = pp.tile([128, 2], F32, tag="bk_sb")
            wq_sb = pp.tile([128, ND * GC], BF16, tag="wq_sb")
            wk_sb = pp.tile([128, ND * GC], BF16, tag="wk_sb")
            wvp_sb = pp.tile([128, ND * VW], BF16, tag="wvp_sb")
            wo_sb = pp.tile([128, 2 * D], BF16, tag="wo_sb")
            dum = pp.tile([128, 512], BF16, tag="dum")
            dscr = pp.tile([128, 16], F32, tag="dscr")

            # ---- all DMAs issued up front -----------------------------
            nc.gpsimd.dma_start(out=bq_sb[:], in_=bq_d[:])
            nc.gpsimd.dma_start(out=bk_sb[:], in_=bk_d[:])
            for d in range(ND):
                nc.sync.dma_start(
                    out=wk_sb[:, d * GC:(d + 1) * GC],
                    in_=wk_d[d * 128:(d + 1) * 128, :])
            for d in range(ND):
                nc.gpsimd.dma_start(
                    out=wvp_sb[:, d * VW:(d + 1) * VW],
                    in_=wvp_d[d * 128:(d + 1) * 128, :])
            # inputs split across both trigger queues; xkv before xq on each
            # (a per-queue gate below enforces it), wq/wo behind the gates
            xkv_tiles = [wk_pool.tile([128, SKV], BF16, tag=f"xkv{d}",
                                      name=f"xkv{d}") for d in range(ND)]
            xkv = xkv_tiles
            xq_tiles = [wk_pool.tile([128, SQ], BF16, tag=f"xq{d}",
                                     name=f"xq{d}") for d in range(ND)]
            for d in range(0, ND, 2):
                nc.sync.dma_start(out=xkv[d][:],
                                  in_=xkvT_d[d * 128:(d + 1) * 128, :])
            for d in range(1, ND, 2):
                nc.gpsimd.dma_start(out=xkv[d][:],
                                    in_=xkvT_d[d * 128:(d + 1) * 128, :])
            # WAR gates: a tiny gpsimd op that reads the last xkv tile of a
            # queue and writes the head of that queue's first xq tile makes
            # the xq DMA trigger wait until xkv is done on that queue
            nc.gpsimd.partition_broadcast(xq_tiles[0][0:1, 0:8],
                                          xkv[6][0:1, 0:8], channels=1)
            nc.gpsimd.partition_broadcast(xq_tiles[1][0:1, 0:8],
                                          xkv[7][0:1, 0:8], channels=1)
            for d in range(0, ND, 2):
                nc.sync.dma_start(out=xq_tiles[d][:],
                                  in_=xqT_d[d * 128:(d + 1) * 128, :])
            for d in range(1, ND, 2):
                nc.gpsimd.dma_start(out=xq_tiles[d][:],
                                    in_=xqT_d[d * 128:(d + 1) * 128, :])
            for d in range(ND):
                (nc.sync if d % 2 == 0 else nc.gpsimd).dma_start(
                    out=wq_sb[:, d * GC:(d + 1) * GC],
                    in_=wq_d[d * 128:(d + 1) * 128, :])
            nc.sync.dma_start(
                out=wo_sb[:].rearrange("p (t n) -> p t n", t=2),
                in_=wo_d.rearrange("(t p) n -> p t n", p=128),
            )

            # ---- warmup: keep the PE busy through the DMA head so the
            # HAM clock gate opens, and pull the ACT exp table load off
            # the critical path
            nc.vector.memset(dum[:], 1.0)
            with tc.tile_pool(name="psW", bufs=1, space="PSUM") as psW:
                wps = psW.tile([128, 512], F32, tag="wps")
                for i in range(16):
                    nc.tensor.matmul(wps[:], dum[:, 0:128], dum[:],
                                     start=True, stop=True)
                nc.scalar.activation(dscr[:], dum[:, 0:16], AF.Exp, scale=1.0)

            # ---- phase 1: KT (d-outer), V' j0-8, QT chunks (0,0),(0,1)
            def qt_drain(ps, p, qc):
                blk = slice(p * SQ + qc * 512, p * SQ + (qc + 1) * 512)
                nc.vector.tensor_scalar_add(
                    qt_sb[:, blk], ps[:], bq_sb[:, p:p + 1])

            vp_done = {}          # j -> pv tile (for filler bookkeeping)

            def vp_copy(j, pv):
                nc.vector.tensor_copy(vp_sb[:, j * VW:(j + 1) * VW], pv[:])
                nc.gpsimd.memset(vp_sb[:, j * VW + 64:(j + 1) * VW:65], 1.0)

            with tc.tile_pool(name="psA", bufs=1, space="PSUM") as psA:
                pk = {}
                for p in range(2):
                    for qc in range(4):
                        pk[p, qc] = psA.tile([128, 512], F32, tag="pk",
                                             bufs=8, name=f"pk{p}{qc}")
                for d in range(ND):
                    for p in range(2):
                        for qc in range(4):
                            nc.tensor.matmul(
                                pk[p, qc][:],
                                wk_sb[:, d * GC + p * 128:d * GC + (p + 1) * 128],
                                xkv[d][:, qc * 512:(qc + 1) * 512],
                                start=(d == 0), stop=(d == ND - 1),
                            )
                for p in range(2):
                    for qc in range(4):
                        nc.scalar.activation(
                            kt_sb[:, p * SKV + qc * 512:p * SKV + (qc + 1) * 512],
                            pk[p, qc][:], AF.Identity, bias=bk_sb[:, p:p + 1])
                # V' j0-8
                for j in range(NJ_PRE):
                    pv = psA.tile([128, VW], F32, tag="pk", bufs=8,
                                  name=f"pv{j}")
                    for d in range(ND):
                        nc.tensor.matmul(
                            pv[:],
                            xkv[d][:, j * 128:(j + 1) * 128],
                            wvp_sb[:, d * VW:(d + 1) * VW],
                            start=(d == 0), stop=(d == ND - 1),
                        )
                    vp_copy(j, pv)
                nc.gpsimd.memset(vp_sb[:, NJ * VW:NJ * VW + 63], 0.0)
                # QT chunk (0,0): just enough Q for the first block
                pq = psA.tile([128, 512], F32, tag="pk", bufs=8, name="pq00")
                for d in range(ND):
                    nc.tensor.matmul(
                        pq[:],
                        wq_sb[:, d * GC:d * GC + 128],
                        xq_tiles[d][:, 0:512],
                        start=(d == 0), stop=(d == ND - 1),
                    )
                nc.scalar.activation(qt_sb[:, 0:512], pq[:],
                                     AF.Identity, bias=bq_sb[:, 0:1])

            # ---- attention with static filler schedule ----------------
            # PSUM: st 2x[128,1024]f32 = 4 banks, o_ps 2x[128,512] = 2,
            # px (V' tail / QT chunks / outproj) 2x[128,512] = 2.
            with (
                tc.tile_pool(name="attn", bufs=1) as at,
                tc.tile_pool(name="psB", bufs=1, space="PSUM") as psB,
            ):
                # --- filler emitters ---------------------------------
                def emit_vp_half(j, half):
                    if half == 0:
                        vp_done[j] = psB.tile([128, VW], F32,
                                              tag=("pxq" if j % 2 else "pxo"),
                                              bufs=1, name=f"pvf{j}")
                    pv = vp_done[j]
                    for d in range(4 * half, 4 * half + 4):
                        nc.tensor.matmul(
                            pv[:],
                            xkv[d][:, j * 128:(j + 1) * 128],
                            wvp_sb[:, d * VW:(d + 1) * VW],
                            start=(d == 0), stop=(d == ND - 1),
                        )
                    if half == 1:
                        vp_copy(j, pv)

                qt_state = {}

                def emit_qt_d(p, qc, d):
                    if d == 0:
                        qt_state[p, qc] = psB.tile([128, 512], F32, tag="pxq",
                                                   bufs=1, name=f"pq{p}{qc}")
                    ps = qt_state[p, qc]
                    nc.tensor.matmul(
                        ps[:],
                        wq_sb[:, d * GC + p * 128:d * GC + (p + 1) * 128],
                        xq_tiles[d][:, qc * 512:(qc + 1) * 512],
                        start=(d == 0), stop=(d == ND - 1),
                    )
                    if d == ND - 1:
                        qt_drain(ps, p, qc)

                def emit_outproj(s, n2, tt, use_act=False):
                    po = psB.tile([128, 512], F32,
                                  tag=("pxo" if (2 * s + n2) % 2 else "pxq"),
                                  bufs=1, name=f"po{s}{n2}{tt}")
                    o_half = o_sbA if s < 8 else o_sbB
                    s8 = s % 8
                    nc.tensor.matmul(
                        po[:],
                        o_half[:, tt * 1024 + s8 * 128:
                               tt * 1024 + (s8 + 1) * 128],
                        wo_sb[:, tt * D + n2 * 512:tt * D + n2 * 512 + 512],
                        start=True, stop=True,
                    )
                    ob = at.tile([128, 512], BF16, tag="ob",
                                 bufs=3, name=f"ob{s}{n2}{tt}")
                    if use_act and (2 * s + n2) % 2 == 0:
                        nc.scalar.copy(ob[:], po[:])
                    else:
                        nc.vector.tensor_copy(ob[:], po[:])
                    (nc.sync if tt == 0 else nc.gpsimd).dma_start(
                        out=out_ds[tt][s * 128:(s + 1) * 128,
                                       n2 * 512:(n2 + 1) * 512],
                        in_=ob[:])

                def opj_items(qq, tt):
                    return [("op", s, n2, tt)
                            for s in range(qq * 4, qq * 4 + 4)
                            for n2 in range(2)]

                def qt_items(p, qc):
                    return [("qt", p, qc, d) for d in range(ND)]

                # per-window filler lists; windows are (t, qq), 16 slots
                fillers = {
                    (0, 0): qt_items(0, 1) + [("vp", j, h)
                                              for j in range(NJ_PRE, NJ)
                                              for h in range(2)],
                    # (1,3) is empty: its opj tiles run in the endgame below

                    (0, 1): qt_items(0, 2) + opj_items(0, 0),
                    (0, 2): qt_items(0, 3) + opj_items(1, 0),
                    (0, 3): qt_items(1, 0) + opj_items(2, 0),
                    (1, 0): qt_items(1, 1) + opj_items(3, 0),
                    (1, 1): qt_items(1, 2) + opj_items(0, 1),
                    (1, 2): qt_items(1, 3) + opj_items(1, 1),
                    (1, 3): [],
                }

                def emit_filler_item(it):
                    kind = it[0]
                    if kind == "vp":
                        emit_vp_half(it[1], it[2])
                    elif kind == "qt":
                        emit_qt_d(it[1], it[2], it[3])
                    else:
                        emit_outproj(it[1], it[2], it[3])

                def emit_av(t, o_ps, p_ts, j):
                    for hp in range(2):
                        h = 2 * t + hp
                        nc.tensor.matmul(
                            o_ps[hp][:],
                            vp_sb[:, j * VW + h * 65:j * VW + h * 65 + 128],
                            p_ts[j][:, hp * 512:(hp + 1) * 512],
                            start=(j == 0), stop=(j == NJ - 1),
                        )

                for t in range(2):          # head pair
                    for qq in range(4):     # q quarter (512)
                        win = list(fillers[(t, qq)])
                        o_ps = {}
                        for hp in range(2):
                            o_ps[hp] = psB.tile(
                                [128, 512], F32, tag="o_ps", bufs=2,
                                name=f"o_ps{t}{qq}{hp}")
                        p_ts = {}
                        for j in range(NJ):
                            st = psB.tile([128, 1024], F32, tag="st",
                                          bufs=2, name=f"st{t}{qq}{j}")
                            for hp in range(2):
                                nc.tensor.matmul(
                                    st[:, hp * 512:(hp + 1) * 512],
                                    kt_sb[hp * 64:(hp + 1) * 64,
                                          t * SKV + j * 128:
                                          t * SKV + (j + 1) * 128],
                                    qt_sb[hp * 64:(hp + 1) * 64,
                                          t * SQ + qq * 512:
                                          t * SQ + (qq + 1) * 512],
                                    start=True, stop=True,
                                )
                            if win:
                                emit_filler_item(win.pop(0))
                            p_ts[j] = at.tile([128, 1024], BF16, tag="pt",
                                              bufs=8, name=f"pt{t}{qq}{j}")
                            nc.scalar.activation(p_ts[j][:], st[:],
                                                 AF.Exp, scale=scale)
                            # AV lags by two kv tiles: the PSUM accumulator
                            # handoff at block starts stays off the PE
                            # critical path
                            if j >= 2:
                                emit_av(t, o_ps, p_ts, j - 2)
                        while win:
                            emit_filler_item(win.pop(0))
                        for j in (NJ - 2, NJ - 1):
                            emit_av(t, o_ps, p_ts, j)
                        if t == 1 and qq == 3:
                            for s, n2 in [(s, n2)
                                          for s in range(8, 12)
                                          for n2 in range(2)]:
                                emit_outproj(s, n2, 1, use_act=True)
                        # normalize: row-sums out first so the
                        # reciprocal+broadcast chain overlaps the O copies
                        o_half = o_sbA if qq < 2 else o_sbB
                        col = t * 1024 + (qq % 2) * 512
                        last_blk = (t == 1 and qq == 3)
                        rs, rcp = {}, {}
                        for hp in range(2):
                            rs[hp] = at.tile([1, 512], F32, tag="rs", bufs=4,
                                             name=f"rs{t}{qq}{hp}")
                            nc.vector.tensor_copy(rs[hp][:],
                                                  o_ps[hp][64:65, :])
                            rcp[hp] = at.tile([1, 512], F32, tag="rcp",
                                              bufs=4, name=f"rcp{t}{qq}{hp}")
                            nc.vector.reciprocal_approx_fast(rcp[hp][:],
                                                             rs[hp][:])
                        ot = {}
                        if not last_blk:
                            for hp in range(2):
                                ot[hp] = at.tile([64, 512], F32, tag="ot",
                                                 bufs=4, name=f"ot{t}{qq}{hp}")
                                nc.vector.tensor_copy(ot[hp][:],
                                                      o_ps[hp][0:64, :])
                        for hp in range(2):
                            bcs = at.tile([64, 512], F32, tag="bcs",
                                          bufs=4, name=f"bcs{t}{qq}{hp}")
                            nc.gpsimd.partition_broadcast(
                                bcs[:], rcp[hp][:], channels=64)
                            nc.vector.tensor_tensor(
                                out=o_half[hp * 64:(hp + 1) * 64,
                                           col:col + 512],
                                in0=(o_ps[hp][0:64, :] if last_blk
                                     else ot[hp][:]),
                                in1=bcs[:], op=MUL)
                # tail: last quarter's second-half outproj
                for s in range(12, 16):
                    for n2 in range(2):
                        emit_outproj(s, n2, 1, use_act=True)

    nc.compile()
    return nc


def build_in_maps(inputs):
    query_input = np.asarray(inputs["query_input"], dtype=np.float32)
    kv_input = np.asarray(inputs["kv_input"], dtype=np.float32)
    Wq = np.asarray(inputs["Wq"], dtype=np.float32)
    bq = np.asarray(inputs["bq"], dtype=np.float32)
    Wkv = np.asarray(inputs["Wkv"], dtype=np.float32)
    bkv = np.asarray(inputs["bkv"], dtype=np.float32)
    Wo = np.asarray(inputs["Wo"], dtype=np.float32)

    Wk = Wkv[:, :D]
    Wv = Wkv[:, D:]
    bk = bkv[:D]

    xT = [np.ascontiguousarray(query_input[b].T).astype(BF16NP) for b in range(B)]
    kvT = [np.ascontiguousarray(kv_input[b].T).astype(BF16NP) for b in range(B)]

    in_maps = []
    for c in range(N_CORES):
        b, g = divmod(c, G)
        c0 = g * GC
        wvp = np.zeros((D, HPG * 65), np.float32)
        for h in range(HPG):
                wvp[:, h * 65:h * 65 + 64] = Wv[:, c0 + h * DH:c0 + (h + 1) * DH]
        bq2 = bq[c0:c0 + GC].reshape(2, 128).T.copy()
        bk2 = bk[c0:c0 + GC].reshape(2, 128).T.copy()
        in_maps.append({
                "xqT": xT[b],
                "xkvT": kvT[b],
                "wq": np.ascontiguousarray(Wq[:, c0:c0 + GC]).astype(BF16NP),
                "wk": np.ascontiguousarray(Wk[:, c0:c0 + GC]).astype(BF16NP),
                "wvp": wvp.astype(BF16NP),
                "wo": np.ascontiguousarray(Wo[c0:c0 + GC, :]).astype(BF16NP),
                "bq2": np.ascontiguousarray(bq2),
                "bk2": np.ascontiguousarray(bk2),
        })
    return in_maps


def kernel(query_input, kv_input, Wq, bq, Wkv, bkv, Wo, bo):
    global _nc_cache
    from concourse import bass_utils

    if _nc_cache is None:
        _nc_cache = _build_nc()
    nc = _nc_cache

    Wkv = np.asarray(Wkv, dtype=np.float32)
    Wo = np.asarray(Wo, dtype=np.float32)
    bo = np.asarray(bo, dtype=np.float32)
    bv = np.asarray(bkv, np.float32)[D:]

    in_maps = build_in_maps(dict(
        query_input=query_input, kv_input=kv_input, Wq=Wq, bq=bq,
        Wkv=Wkv, bkv=bkv, Wo=Wo))

    res = bass_utils.run_bass_kernel_spmd(nc, in_maps,
                                          core_ids=list(range(N_CORES)))

    # gather: sum the 8 head-pair partials per batch; add biases the device
    # left out (bo, and bv which passes through Wo since softmax rows sum to 1)
    tail = bv @ Wo + bo
    out = np.empty((B, SQ, D), np.float32)
    for b in range(B):
        acc = res.results[b * G + 0]["out_p0"].astype(np.float32).copy()
        acc += res.results[b * G + 0]["out_p1"]
        for g in range(1, G):
                acc += res.results[b * G + g]["out_p0"]
                acc += res.results[b * G + g]["out_p1"]
        out[b] = acc + tail[None, :]
    return out


# revision 20
# speedup vs baseline: 1.0240x; 1.0240x over previous
"""Multi-head cross attention on 8 Trainium2 NeuronCores.

Sharding: core c = b*4 + g handles batch b (of 2) and head-group g (4 heads
of the 16).  Each core projects Q/K/V for its 4 heads, runs attention, and
computes partial output projections with its 256 rows of Wo split into two
128-row head-pair chunks; the host sums the 8 bf16 partials per batch (plus
bo and the bv@Wo term, exact because softmax rows sum to 1).

v3 schedule: one software-pipelined stream.  All input DMAs are issued up
front (weights, then xkv, then xq); KT matmuls chase the xkv tiles as they
land; V' runs for kv tiles 0-8 and the first two Q chunks, then attention
starts (~45us in).  Everything else - the remaining V' tiles, the remaining
QT chunks, and all output-projection tiles - runs as PE filler inside the
ACT-bound attention loop (exp gates it at ~1.2us per kv tile).  AV matmuls
lag the score matmuls by two kv tiles so the PSUM-accumulator handoff at
block boundaries never stalls the PE queue ahead of the exp stream.  The
output projection is split by head pair (one K=128 matmul per tile) so each
half runs as soon as its pair is normalized; only the last quarter's second
half remains as tail.  Row-sum reciprocals use the fast custom-DVE op.
"""

import sys

sys.path.insert(0, "/opt/trn_rl_repo")

import ml_dtypes
import numpy as np

BF16NP = ml_dtypes.bfloat16

B, SQ, SKV, D, H = 2, 2048, 2048, 1024, 16
DH = D // H          # 64
N_CORES = 8
G = 4                # head groups
HPG = H // G         # heads per group = 4
GC = HPG * DH        # group width = 256

_nc_cache = None


def _build_nc():
    import concourse.mybir as mybir
    import concourse.tile as tile
    from concourse import bacc

    F32 = mybir.dt.float32
    BF16 = mybir.dt.bfloat16
    AF = mybir.ActivationFunctionType
    MUL = mybir.AluOpType.mult

    nc = bacc.Bacc("TRN2", target_bir_lowering=False, debug=False,
                   num_devices=N_CORES)

    xqT_d = nc.dram_tensor("xqT", [D, SQ], BF16, kind="ExternalInput").ap()
    xkvT_d = nc.dram_tensor("xkvT", [D, SKV], BF16, kind="ExternalInput").ap()
    wq_d = nc.dram_tensor("wq", [D, GC], BF16, kind="ExternalInput").ap()
    wk_d = nc.dram_tensor("wk", [D, GC], BF16, kind="ExternalInput").ap()
    # Wv' with a zero column after each head's 64 (slots for the ones column)
    wvp_d = nc.dram_tensor("wvp", [D, HPG * 65], BF16, kind="ExternalInput").ap()
    wo_d = nc.dram_tensor("wo", [GC, D], BF16, kind="ExternalInput").ap()
    bq_d = nc.dram_tensor("bq2", [128, 2], F32, kind="ExternalInput").ap()
    bk_d = nc.dram_tensor("bk2", [128, 2], F32, kind="ExternalInput").ap()
    out0_d = nc.dram_tensor("out_p0", [SQ, D], BF16, kind="ExternalOutput").ap()
    out1_d = nc.dram_tensor("out_p1", [SQ, D], BF16, kind="ExternalOutput").ap()
    out_ds = [out0_d, out1_d]

    ND = D // 128        # 8 d-tiles (contraction over D)
    NJ = SKV // 128      # 16 kv tiles
    VW = HPG * 65        # 260, V' row width
    NJ_PRE = 10          # V' tiles computed before attention starts
    scale = 1.0 / float(np.sqrt(DH))

    with tile.TileContext(nc) as tc:
        with (
            tc.tile_pool(name="persist", bufs=1) as pp,
            tc.tile_pool(name="work", bufs=1) as wk_pool,
        ):
            # ---- persistent tiles -------------------------------------
            qt_sb = pp.tile([128, 2 * SQ], BF16, tag="qt_sb")
            kt_sb = pp.tile([128, 2 * SKV], BF16, tag="kt_sb")
            vp_sb = pp.tile([128, NJ * VW + 63], BF16, tag="vp_sb")
            o_sbA = pp.tile([128, 2 * 1024], BF16, tag="o_sbA")
            o_sbB = pp.tile([128, 2 * 1024], BF16, tag="o_sbB")
            bq_sb = pp.tile([128, 2], F32, tag="bq_sb")
            bk_sb = pp.tile([128, 2], F32, tag="bk_sb")
            wq_sb = pp.tile([128, ND * GC], BF16, tag="wq_sb")
            wk_sb = pp.tile([128, ND * GC], BF16, tag="wk_sb")
            wvp_sb = pp.tile([128, ND * VW], BF16, tag="wvp_sb")
            wo_sb = pp.tile([128, 2 * D], BF16, tag="wo_sb")
            dum = pp.tile([128, 512], BF16, tag="dum")
            dscr = pp.tile([128, 16], F32, tag="dscr")

            # ---- all DMAs issued up front -----------------------------
            nc.gpsimd.dma_start(out=bq_sb[:], in_=bq_d[:])
            nc.gpsimd.dma_start(out=bk_sb[:], in_=bk_d[:])
            for d in range(ND):
                nc.sync.dma_start(
                    out=wk_sb[:, d * GC:(d + 1) * GC],
                    in_=wk_d[d * 128:(d + 1) * 128, :])
            for d in range(ND):
                nc.gpsimd.dma_start(
                    out=wvp_sb[:, d * VW:(d + 1) * VW],
                    in_=wvp_d[d * 128:(d + 1) * 128, :])
            # inputs split across both trigger queues; xkv before xq on each
            # (a per-queue gate below enforces it), wq/wo behind the gates
            xkv_tiles = [wk_pool.tile([128, SKV], BF16, tag=f"xkv{d}",
                                      name=f"xkv{d}") for d in range(ND)]
            xkv = xkv_tiles
            xq_tiles = [wk_pool.tile([128, SQ], BF16, tag=f"xq{d}",
                                     name=f"xq{d}") for d in range(ND)]
            for d in range(0, ND, 2):
                nc.sync.dma_start(out=xkv[d][:],
                                  in_=xkvT_d[d * 128:(d + 1) * 128, :])
            for d in range(1, ND, 2):
                nc.gpsimd.dma_start(out=xkv[d][:],
                                    in_=xkvT_d[d * 128:(d + 1) * 128, :])
            # WAR gates: a tiny gpsimd op that reads the last xkv tile of a
            # queue and writes the head of that queue's first xq tile makes
            # the xq DMA trigger wait until xkv is done on that queue
            nc.gpsimd.partition_broadcast(xq_tiles[0][0:1, 0:8],
                                          xkv[6][0:1, 0:8], channels=1)
            nc.gpsimd.partition_broadcast(xq_tiles[1][0:1, 0:8],
                                          xkv[7][0:1, 0:8], channels=1)
            for d in range(0, ND, 2):
                nc.sync.dma_start(out=xq_tiles[d][:],
                                  in_=xqT_d[d * 128:(d + 1) * 128, :])
            for d in range(1, ND, 2):
                nc.gpsimd.dma_start(out=xq_tiles[d][:],
                                    in_=xqT_d[d * 128:(d + 1) * 128, :])
            for d in range(ND):
                (nc.sync if d % 2 == 0 else nc.gpsimd).dma_start(
                    out=wq_sb[:, d * GC:(d + 1) * GC],
                    in_=wq_d[d * 128:(d + 1) * 128, :])
            nc.sync.dma_start(
                out=wo_sb[:].rearrange("p (t n) -> p t n", t=2),
                in_=wo_d.rearrange("(t p) n -> p t n", p=128),
            )

            # ---- warmup: keep the PE busy through the DMA head so the
            # HAM clock gate opens, and pull the ACT exp table load off
            # the critical path
            nc.vector.memset(dum[:], 1.0)
            with tc.tile_pool(name="psW", bufs=1, space="PSUM") as psW:
                wps = psW.tile([128, 512], F32, tag="wps")
                for i in range(16):
                    nc.tensor.matmul(wps[:], dum[:, 0:128], dum[:],
                                     start=True, stop=True)
                nc.scalar.activation(dscr[:], dum[:, 0:16], AF.Exp, scale=1.0)

            # ---- phase 1: KT (d-outer), V' j0-8, QT chunks (0,0),(0,1)
            def qt_drain(ps, p, qc):
                blk = slice(p * SQ + qc * 512, p * SQ + (qc + 1) * 512)
                nc.vector.tensor_scalar_add(
                    qt_sb[:, blk], ps[:], bq_sb[:, p:p + 1])

            vp_done = {}          # j -> pv tile (for filler bookkeeping)

            def vp_copy(j, pv):
                nc.vector.tensor_copy(vp_sb[:, j * VW:(j + 1) * VW], pv[:])
                nc.gpsimd.memset(vp_sb[:, j * VW + 64:(j + 1) * VW:65], 1.0)

            with tc.tile_pool(name="psA", bufs=1, space="PSUM") as psA:
                pk = {}
                for p in range(2):
                    for qc in range(4):
                        pk[p, qc] = psA.tile([128, 512], F32, tag="pk",
                                             bufs=8, name=f"pk{p}{qc}")
                for d in range(ND):
                    for p in range(2):
                        for qc in range(4):
                            nc.tensor.matmul(
                                pk[p, qc][:],
                                wk_sb[:, d * GC + p * 128:d * GC + (p + 1) * 128],
                                xkv[d][:, qc * 512:(qc + 1) * 512],
                                start=(d == 0), stop=(d == ND - 1),
                            )
                for p in range(2):
                    for qc in range(4):
                        nc.scalar.activation(
                            kt_sb[:, p * SKV + qc * 512:p * SKV + (qc + 1) * 512],
                            pk[p, qc][:], AF.Identity, bias=bk_sb[:, p:p + 1])
                # V' j0-8
                for j in range(NJ_PRE):
                    pv = psA.tile([128, VW], F32, tag="pk", bufs=8,
                                  name=f"pv{j}")
                    for d in range(ND):
                        nc.tensor.matmul(
                            pv[:],
                            xkv[d][:, j * 128:(j + 1) * 128],
                            wvp_sb[:, d * VW:(d + 1) * VW],
                            start=(d == 0), stop=(d == ND - 1),
                        )
                    vp_copy(j, pv)
                nc.gpsimd.memset(vp_sb[:, NJ * VW:NJ * VW + 63], 0.0)
                # QT chunk (0,0): just enough Q for the first block
                pq = psA.tile([128, 512], F32, tag="pk", bufs=8, name="pq00")
                for d in range(ND):
                    nc.tensor.matmul(
                        pq[:],
                        wq_sb[:, d * GC:d * GC + 128],
                        xq_tiles[d][:, 0:512],
                        start=(d == 0), stop=(d == ND - 1),
                    )
                nc.scalar.activation(qt_sb[:, 0:512], pq[:],
                                     AF.Identity, bias=bq_sb[:, 0:1])

            # ---- attention with static filler schedule ----------------
            # PSUM: st 2x[128,1024]f32 = 4 banks, o_ps 2x[128,512] = 2,
            # px (V' tail / QT chunks / outproj) 2x[128,512] = 2.
            with (
                tc.tile_pool(name="attn", bufs=1) as at,
                tc.tile_pool(name="psB", bufs=1, space="PSUM") as psB,
            ):
                # --- filler emitters ---------------------------------
                def emit_vp_half(j, half):
                    if half == 0:
                        vp_done[j] = psB.tile([128, VW], F32,
                                              tag=("pxq" if j % 2 else "pxo"),
                                              bufs=1, name=f"pvf{j}")
                    pv = vp_done[j]
                    for d in range(4 * half, 4 * half + 4):
                        nc.tensor.matmul(
                            pv[:],
                            xkv[d][:, j * 128:(j + 1) * 128],
                            wvp_sb[:, d * VW:(d + 1) * VW],
                            start=(d == 0), stop=(d == ND - 1),
                        )
                    if half == 1:
                        vp_copy(j, pv)

                qt_state = {}

                def emit_qt_d(p, qc, d):
                    if d == 0:
                        qt_state[p, qc] = psB.tile([128, 512], F32, tag="pxq",
                                                   bufs=1, name=f"pq{p}{qc}")
                    ps = qt_state[p, qc]
                    nc.tensor.matmul(
                        ps[:],
                        wq_sb[:, d * GC + p * 128:d * GC + (p + 1) * 128],
                        xq_tiles[d][:, qc * 512:(qc + 1) * 512],
                        start=(d == 0), stop=(d == ND - 1),
                    )
                    if d == ND - 1:
                        qt_drain(ps, p, qc)

                def emit_outproj(s, n2, tt):
                    po = psB.tile([128, 512], F32,
                                  tag=("pxo" if (2 * s + n2) % 2 else "pxq"),
                                  bufs=1, name=f"po{s}{n2}{tt}")
                    o_half = o_sbA if s < 8 else o_sbB
                    s8 = s % 8
                    nc.tensor.matmul(
                        po[:],
                        o_half[:, tt * 1024 + s8 * 128:
                               tt * 1024 + (s8 + 1) * 128],
                        wo_sb[:, tt * D + n2 * 512:tt * D + n2 * 512 + 512],
                        start=True, stop=True,
                    )
                    ob = at.tile([128, 512], BF16, tag="ob",
                                 bufs=3, name=f"ob{s}{n2}{tt}")
                    nc.vector.tensor_copy(ob[:], po[:])
                    (nc.sync if tt == 0 else nc.gpsimd).dma_start(
                        out=out_ds[tt][s * 128:(s + 1) * 128,
                                       n2 * 512:(n2 + 1) * 512],
                        in_=ob[:])

                def opj_items(qq, tt):
                    return [("op", s, n2, tt)
                            for s in range(qq * 4, qq * 4 + 4)
                            for n2 in range(2)]

                def qt_items(p, qc):
                    return [("qt", p, qc, d) for d in range(ND)]

                # per-window filler lists; windows are (t, qq), 16 slots
                fillers = {
                    (0, 0): qt_items(0, 1) + [("vp", j, h)
                                              for j in range(NJ_PRE, NJ)
                                              for h in range(2)],
                    # (1,3) is empty: its opj tiles run in the endgame below

                    (0, 1): qt_items(0, 2) + opj_items(0, 0),
                    (0, 2): qt_items(0, 3) + opj_items(1, 0),
                    (0, 3): qt_items(1, 0) + opj_items(2, 0),
                    (1, 0): qt_items(1, 1) + opj_items(3, 0),
                    (1, 1): qt_items(1, 2) + opj_items(0, 1),
                    (1, 2): qt_items(1, 3) + opj_items(1, 1),
                    (1, 3): opj_items(2, 1),
                }

                def emit_filler_item(it):
                    kind = it[0]
                    if kind == "vp":
                        emit_vp_half(it[1], it[2])
                    elif kind == "qt":
                        emit_qt_d(it[1], it[2], it[3])
                    else:
                        emit_outproj(it[1], it[2], it[3])

                def emit_av(t, o_ps, p_ts, j):
                    for hp in range(2):
                        h = 2 * t + hp
                        nc.tensor.matmul(
                            o_ps[hp][:],
                            vp_sb[:, j * VW + h * 65:j * VW + h * 65 + 128],
                            p_ts[j][:, hp * 512:(hp + 1) * 512],
                            start=(j == 0), stop=(j == NJ - 1),
                        )

                for t in range(2):          # head pair
                    for qq in range(4):     # q quarter (512)
                        win = list(fillers[(t, qq)])
                        o_ps = {}
                        for hp in range(2):
                            o_ps[hp] = psB.tile(
                                [128, 512], F32, tag="o_ps", bufs=2,
                                name=f"o_ps{t}{qq}{hp}")
                        p_ts = {}
                        for j in range(NJ):
                            st = psB.tile([128, 1024], F32, tag="st",
                                          bufs=2, name=f"st{t}{qq}{j}")
                            for hp in range(2):
                                nc.tensor.matmul(
                                    st[:, hp * 512:(hp + 1) * 512],
                                    kt_sb[hp * 64:(hp + 1) * 64,
                                          t * SKV + j * 128:
                                          t * SKV + (j + 1) * 128],
                                    qt_sb[hp * 64:(hp + 1) * 64,
                                          t * SQ + qq * 512:
                                          t * SQ + (qq + 1) * 512],
                                    start=True, stop=True,
                                )
                            if win:
                                emit_filler_item(win.pop(0))
                            p_ts[j] = at.tile([128, 1024], BF16, tag="pt",
                                              bufs=8, name=f"pt{t}{qq}{j}")
                            nc.scalar.activation(p_ts[j][:], st[:],
                                                 AF.Exp, scale=scale)
                            # AV lags by two kv tiles: the PSUM accumulator
                            # handoff at block starts stays off the PE
                            # critical path
                            if j >= 2:
                                emit_av(t, o_ps, p_ts, j - 2)
                        while win:
                            emit_filler_item(win.pop(0))
                        for j in (NJ - 2, NJ - 1):
                            emit_av(t, o_ps, p_ts, j)
                        # normalize: row-sums out first so the
                        # reciprocal+broadcast chain overlaps the O copies
                        o_half = o_sbA if qq < 2 else o_sbB
                        col = t * 1024 + (qq % 2) * 512
                        last_blk = (t == 1 and qq == 3)
                        rs, rcp = {}, {}
                        for hp in range(2):
                            rs[hp] = at.tile([1, 512], F32, tag="rs", bufs=4,
                                             name=f"rs{t}{qq}{hp}")
                            nc.vector.tensor_copy(rs[hp][:],
                                                  o_ps[hp][64:65, :])
                            rcp[hp] = at.tile([1, 512], F32, tag="rcp",
                                              bufs=4, name=f"rcp{t}{qq}{hp}")
                            nc.vector.reciprocal_approx_fast(rcp[hp][:],
                                                             rs[hp][:])
                        ot = {}
                        if not last_blk:
                            for hp in range(2):
                                ot[hp] = at.tile([64, 512], F32, tag="ot",
                                                 bufs=4, name=f"ot{t}{qq}{hp}")
                                nc.vector.tensor_copy(ot[hp][:],
                                                      o_ps[hp][0:64, :])
                        for hp in range(2):
                            bcs = at.tile([64, 512], F32, tag="bcs",
                                          bufs=4, name=f"bcs{t}{qq}{hp}")
                            nc.gpsimd.partition_broadcast(
                                bcs[:], rcp[hp][:], channels=64)
                            nc.vector.tensor_tensor(
                                out=o_half[hp * 64:(hp + 1) * 64,
                                           col:col + 512],
                                in0=(o_ps[hp][0:64, :] if last_blk
                                     else ot[hp][:]),
                                in1=bcs[:], op=MUL)
                # tail: last quarter's second-half outproj
                for it in opj_items(3, 1):
                    emit_filler_item(it)

    nc.compile()
    return nc


def build_in_maps(inputs):
    query_input = np.asarray(inputs["query_input"], dtype=np.float32)
    kv_input = np.asarray(inputs["kv_input"], dtype=np.float32)
    Wq = np.asarray(inputs["Wq"], dtype=np.float32)
    bq = np.asarray(inputs["bq"], dtype=np.float32)
    Wkv = np.asarray(inputs["Wkv"], dtype=np.float32)
    bkv = np.asarray(inputs["bkv"], dtype=np.float32)
    Wo = np.asarray(inputs["Wo"], dtype=np.float32)

    Wk = Wkv[:, :D]
    Wv = Wkv[:, D:]
    bk = bkv[:D]

    xT = [np.ascontiguousarray(query_input[b].T).astype(BF16NP) for b in range(B)]
    kvT = [np.ascontiguousarray(kv_input[b].T).astype(BF16NP) for b in range(B)]

    in_maps = []
    for c in range(N_CORES):
        b, g = divmod(c, G)
        c0 = g * GC
        wvp = np.zeros((D, HPG * 65), np.float32)
        for h in range(HPG):
                wvp[:, h * 65:h * 65 + 64] = Wv[:, c0 + h * DH:c0 + (h + 1) * DH]
        bq2 = bq[c0:c0 + GC].reshape(2, 128).T.copy()
        bk2 = bk[c0:c0 + GC].reshape(2, 128).T.copy()
        in_maps.append({
                "xqT": xT[b],
                "xkvT": kvT[b],
                "wq": np.ascontiguousarray(Wq[:, c0:c0 + GC]).astype(BF16NP),
                "wk": np.ascontiguousarray(Wk[:, c0:c0 + GC]).astype(BF16NP),
                "wvp": wvp.astype(BF16NP),
                "wo": np.ascontiguousarray(Wo[c0:c0 + GC, :]).astype(BF16NP),
                "bq2": np.ascontiguousarray(bq2),
                "bk2": np.ascontiguousarray(bk2),
        })
    return in_maps


def kernel(query_input, kv_input, Wq, bq, Wkv, bkv, Wo, bo):
    global _nc_cache
    from concourse import bass_utils

    if _nc_cache is None:
        _nc_cache = _build_nc()
    nc = _nc_cache

    Wkv = np.asarray(Wkv, dtype=np.float32)
    Wo = np.asarray(Wo, dtype=np.float32)
    bo = np.asarray(bo, dtype=np.float32)
    bv = np.asarray(bkv, np.float32)[D:]

    in_maps = build_in_maps(dict(
        query_input=query_input, kv_input=kv_input, Wq=Wq, bq=bq,
        Wkv=Wkv, bkv=bkv, Wo=Wo))

    res = bass_utils.run_bass_kernel_spmd(nc, in_maps,
                                          core_ids=list(range(N_CORES)))

    # gather: sum the 8 head-pair partials per batch; add biases the device
    # left out (bo, and bv which passes through Wo since softmax rows sum to 1)
    tail = bv @ Wo + bo
    out = np.empty((B, SQ, D), np.float32)
    for b in range(B):
        acc = res.results[b * G + 0]["out_p0"].astype(np.float32).copy()
        acc += res.results[b * G + 0]["out_p1"]
        for g in range(1, G):
                acc += res.results[b * G + g]["out_p0"]
                acc += res.results[b * G + g]["out_p1"]
        out[b] = acc + tail[None, :]
    return out
